# revision 1
# baseline (speedup 1.0000x reference)
"""Trainium2 Bass kernel v2 for nn_Attention_22050362097758 (edge-biased causal
attention; b=2, n=512, dim=256, heads=8, dim_head=64).

Sharding: core c -> batch c//4, lane l=c%4, query rows i = 4r+l (r=0..127).
Row-interleaving makes causal extents core-uniform (SPMD: one program).

Edges path (the bulk): host pre-casts edges to bf16 and packs each row's
causal prefix (padded to 128-col tiles) contiguously. Device: SWDGE load
[128 j, jt, 256 d] -> ONE xbar transpose-DMA per row -> eT [128 d, 2jt, 128 j]
-> per 128-j tile: bias^T[j,h] = eT^T @ (gamma_e-folded W_edge) on PE;
ss = sum(e^2) via one fused DVE scalar_tensor_tensor (ACT Square for a
subset, to balance engines); rsqrt batched 16 tiles on ACT; staging write
fuses rinv scale + causal/key mask add (tensor_scalar, two scalars).

Attention in j-on-partitions layout: sim^T[j,i] = kT_blk^T @ qT per (h,
j-block); + staging (bias+mask) -> exp (b_edge via ACT bias) -> denominators
by ones-matmul, attn @ V with attn^T as lhsT (no transposes); out proj via
one xbar transpose-DMA + 4 matmuls. Rows emitted descending so sim phase
(j-block t needs rows r>=32t) overlaps the edge stream.
"""
import sys
sys.path.insert(0, "/opt/trn_rl_repo")
import numpy as np
import ml_dtypes

import concourse.bass as bass
import concourse.mybir as mybir
import concourse.tile as tile
from concourse.bass_utils import run_bass_kernel_spmd

B, N, DIM = 2, 512, 256
H, DH = 8, 64
EPS = 1e-5
NEG = -1e30
F32 = mybir.dt.float32
BF16 = mybir.dt.bfloat16
BF = ml_dtypes.bfloat16

# Tiles enumerated ascending (r, t) — matches the DRAM packing order, so
# a 16-tile chunk is one contiguous 1MB slab of ef. Chunks emitted
# descending (high rows first) so sim j-block t can start as soon as rows
# >= 32t are staged.
TILE_IDX = {}
TILE_REV = []
for _r in range(128):
    for _t in range(_r // 32 + 1):
        TILE_IDX[(_r, _t)] = len(TILE_REV)
        TILE_REV.append((_r, _t))
NT = len(TILE_REV)   # 320
NCH = NT // 16       # 20 chunks
EF_ROWS = NT * 128   # 40960

# ss/rinv/mask columns laid out t-grouped: block t occupies columns
# [TOFF[t], TOFF[t+1]) indexed by (r - 32t), so per-block slices are
# contiguous [128 j, n_rows] matrices usable directly in broadcast TTs.
TOFF = [0, 128, 224, 288, 320]
NR = [128, 96, 64, 32]
def POS(r, t):
    return TOFF[t] + (r - 32 * t)

# per-chunk group structure: groups[t] = (sig0, q_list, r0, cnt)
CHUNK_GROUPS = []
for _c in range(NCH):
    by_t = {}
    for _q in range(16):
        _r, _t = TILE_REV[16 * _c + _q]
        by_t.setdefault(_t, []).append((_q, _r))
    groups = []
    sig = 0
    for _t in sorted(by_t):
        qs = by_t[_t]
        r0 = qs[0][1]
        assert [r for _, r in qs] == list(range(r0, r0 + len(qs)))
        groups.append((_t, sig, [q for q, _ in qs], r0, len(qs)))
        sig += len(qs)
    CHUNK_GROUPS.append(groups)

# ss engine split by k%16: 'a'=ACT Square, 'g'=GPSIMD square + DVE reduce,
# 'v'=DVE fused scalar_tensor_tensor
SS_PATTERN = "vavavvavvavvavav"

_ctr = [0]


def _nop_with_wait(engine, wait):
    _ctr[0] += 1
    n = mybir.InstNoOp.__new__(mybir.InstNoOp, name=f"waitnop-{_ctr[0]}")
    n.engine = engine
    n.sync_info = mybir.SyncInfo.__new__(mybir.SyncInfo, on_wait=[wait], on_update=[])
    return n


def split_waits(nc):
    """Walrus encodes at most ONE sem-wait per instruction; Tile attaches
    many. Move extras onto NOPs inserted just before, same engine."""
    for f in nc.m.functions:
        for b in f.blocks:
            out, changed = [], False
            for inst in b.instructions:
                si = inst.sync_info
                waits = list(si.on_wait) if (si and si.on_wait) else []
                keep = 0 if inst.opcode == "Drain" else 1
                if len(waits) > keep:
                    changed = True
                    moved = waits[:-keep] if keep else waits
                    kept = waits[-keep:] if keep else []
                    for w in moved:
                        out.append(_nop_with_wait(inst.engine, w))
                    inst.sync_info = mybir.SyncInfo.__new__(
                        mybir.SyncInfo, on_wait=kept,
                        on_update=list(si.on_update) if si.on_update else [])
                out.append(inst)
            if changed:
                b.instructions = out


def build(debug=False):
    nc = bass.Bass()
    ef_ext = nc.declare_dram_parameter("ef", [EF_ROWS, DIM], BF16, isOutput=False)
    x_ext = nc.declare_dram_parameter("xb", [N, DIM], F32, isOutput=False)
    xq_ext = nc.declare_dram_parameter("xq", [128, DIM], F32, isOutput=False)
    wq32_ext = nc.declare_dram_parameter("wq32", [128, 2, 8, 128], F32, isOutput=False)
    wv16_ext = nc.declare_dram_parameter("wv16", [128, 2, 4, 128], BF16, isOutput=False)
    we16_ext = nc.declare_dram_parameter("we16", [128, 2, H], BF16, isOutput=False)
    wo16_ext = nc.declare_dram_parameter("wo16", [128, 4, DIM], BF16, isOutput=False)
    mcol_ext = nc.declare_dram_parameter("mcol", [128, NT], F32, isOutput=False)
    bedge_ext = nc.declare_dram_parameter("bedgec", [128, H], F32, isOutput=False)
    id_ext = nc.declare_dram_parameter("ident", [128, 128], BF16, isOutput=False)
    out_ext = nc.declare_dram_parameter("out", [128, DIM], F32, isOutput=True)
    if debug:
        dbg_stg = [nc.declare_dram_parameter(f"dbg_stg{t}", [128, NR[t], H], F32,
                                             isOutput=True) for t in range(4)]
        dbg_ss = nc.declare_dram_parameter("dbg_ss", [128, NT], F32, isOutput=True)
        dbg_rinv = nc.declare_dram_parameter("dbg_rinv", [128, NT], F32, isOutput=True)
        dbg_den = nc.declare_dram_parameter("dbg_den", [1, 2, 512], F32, isOutput=True)
        dbg_av = nc.declare_dram_parameter("dbg_av", [128, H * DH], F32, isOutput=True)
        dbg_kT = nc.declare_dram_parameter("dbg_kT", [128, 4, N], F32, isOutput=True)
        dbg_qT = nc.declare_dram_parameter("dbg_qT", [128, 4, 128], F32, isOutput=True)
        dbg_raw = [nc.declare_dram_parameter(f"dbg_raw{t}", [128, NR[t], H], F32,
                                             isOutput=True) for t in range(4)]

    AF = mybir.ActivationFunctionType
    MUL, ADD = mybir.AluOpType.mult, mybir.AluOpType.add

    with tile.TileContext(nc) as tc:
        with tc.tile_pool(name="cst", bufs=1) as cst, \
             tc.tile_pool(name="ep", bufs=2) as ep, \
             tc.tile_pool(name="etp", bufs=2) as etp, \
             tc.tile_pool(name="sqp", bufs=2) as sqp, \
             tc.tile_pool(name="scrp", bufs=3) as scrp, \
             tc.tile_pool(name="attnp", bufs=3) as attnp, \
             tc.tile_pool(name="wk", bufs=2) as wk, \
             tc.tile_pool(name="bps", bufs=2, space="PSUM") as bps, \
             tc.tile_pool(name="sps", bufs=2, space="PSUM") as sps, \
             tc.tile_pool(name="dps", bufs=1, space="PSUM") as dps, \
             tc.tile_pool(name="mps", bufs=1, space="PSUM") as mps, \
             tc.tile_pool(name="avps", bufs=1, space="PSUM") as avps:

            # ---------------- constants ----------------
            ident = cst.tile([128, 128], BF16)
            nc.sync.dma_start(out=ident, in_=id_ext[:, :])
            we16 = cst.tile([128, 2, H], BF16)
            nc.sync.dma_start(out=we16, in_=we16_ext[:, :, :])
            wo16 = cst.tile([128, 4, DIM], BF16)
            nc.sync.dma_start(out=wo16, in_=wo16_ext[:, :, :])
            wq32 = cst.tile([128, 2, 8, 128], F32)
            nc.sync.dma_start(out=wq32, in_=wq32_ext[:, :, :, :])
            wv16 = cst.tile([128, 2, 4, 128], BF16)
            nc.sync.dma_start(out=wv16, in_=wv16_ext[:, :, :, :])
            mcol = cst.tile([128, NT], F32)
            nc.sync.dma_start(out=mcol, in_=mcol_ext[:, :])
            bedge = cst.tile([128, H], F32)
            nc.sync.dma_start(out=bedge, in_=bedge_ext[:, :])
            x32 = cst.tile([128, 5, DIM], F32)
            nc.sync.dma_start(out=x32[:, 0:4, :],
                              in_=x_ext.rearrange("(t p) d -> p t d", p=128))
            nc.sync.dma_start(out=x32[:, 4, :], in_=xq_ext[:, :])

            epsc = cst.tile([128, 1], F32)
            nc.vector.memset(epsc, EPS)
            ones16 = cst.tile([128, 1], BF16)
            nc.vector.memset(ones16, 1.0)
            onef = cst.tile([1, 1], F32)
            nc.vector.memset(onef, 1.0)

            ss_all = cst.tile([128, NT], F32)
            rinv_all = cst.tile([128, NT], F32)
            stgraw = []
            stgfin = []
            for t in range(4):
                sr = cst.tile([128, NR[t], H], F32, tag=f"sraw{t}", name=f"sraw{t}")
                stgraw.append(sr)
                sf = cst.tile([128, NR[t], H], F32, tag=f"sfin{t}", name=f"sfin{t}")
                stgfin.append(sf)
            stgtmp = cst.tile([128, 128, H], F32)

            # ---------------- x path: rmsnorm, kT, qT, v ----------------
            ssx = wk.tile([128, 5], F32, tag="ssx")
            for t in range(5):
                dump = wk.tile([128, DIM], BF16, tag="sqdump")
                nc.scalar.activation(out=dump, in_=x32[:, t, :],
                                     func=AF.Square, accum_out=ssx[:, t:t + 1])
            sqm = wk.tile([128, 5], F32, tag="sqm")
            nc.scalar.activation(out=sqm, in_=ssx, func=AF.Ln,
                                 bias=epsc, scale=1.0 / DIM)
            rx = wk.tile([128, 5], F32, tag="rx")
            nc.scalar.activation(out=rx, in_=sqm, func=AF.Exp, scale=-0.5)
            xn32 = cst.tile([128, 5, DIM], F32)
            for t in range(5):
                nc.vector.tensor_scalar(out=xn32[:, t, :], in0=x32[:, t, :],
                                        scalar1=rx[:, t:t + 1], scalar2=None,
                                        op0=MUL)
            ident32 = cst.tile([128, 128], F32)
            nc.vector.tensor_copy(ident32, ident)
            xnT32 = cst.tile([128, 2, 5, 128], F32)
            for t in range(5):
                ps32 = mps.tile([128, 2, 128], F32, tag="setup_ps")
                for kh in range(2):
                    nc.tensor.transpose(ps32[:, kh, :],
                                        xn32[:, t, kh * 128:(kh + 1) * 128], ident32)
                nc.vector.tensor_copy(xnT32[:, :, t, :], ps32)
            xnT = cst.tile([128, 2, 5, 128], BF16)
            nc.vector.tensor_copy(xnT, xnT32)

            kT = cst.tile([128, 4, N], F32)
            for ft in range(4):
                k_ps = mps.tile([128, N], F32, tag="setup_ps")
                for kh in range(2):
                    nc.tensor.matmul(k_ps,
                                     lhsT=wq32[:, kh, 4 + ft, :],
                                     rhs=xnT32[:, kh, 0:4, :].rearrange("p a b -> p (a b)"),
                                     start=(kh == 0), stop=(kh == 1))
                nc.vector.tensor_copy(kT[:, ft, :], k_ps)
            qT = cst.tile([128, 4, 128], F32)
            for ft in range(4):
                q_ps = mps.tile([128, 128], F32, tag="setup_ps")
                for kh in range(2):
                    nc.tensor.matmul(q_ps, lhsT=wq32[:, kh, ft, :],
                                     rhs=xnT32[:, kh, 4, :],
                                     start=(kh == 0), stop=(kh == 1))
                nc.vector.tensor_copy(qT[:, ft, :], q_ps)
            v16 = cst.tile([128, 4, H * DH], BF16)
            for st in range(4):
                v_ps = mps.tile([128, H * DH], F32, tag="setup_ps")
                for kh in range(2):
                    nc.tensor.matmul(v_ps,
                                     lhsT=xnT[:, kh, st, :],
                                     rhs=wv16[:, kh, :, :].rearrange("p a b -> p (a b)"),
                                     start=(kh == 0), stop=(kh == 1))
                nc.vector.tensor_copy(v16[:, st, :], v_ps)

            av_ps = avps.tile([128, H * DH], F32, tag="avout")
            den_ps = []
            for g in range(2):
                dtile = dps.tile([1, 512], F32, tag=f"den{g}", name=f"den_ps{g}")
                den_ps.append(dtile)
            # attn tiles stored in SBUF; den/av accumulation chains run
            # contiguously at the end (PSUM has_written clears are bank-wide
            # on start=True, so interleaved per-h chains lose earlier sums).
            attn_all = cst.tile([128, 4, H, 128], BF16)
            nc.gpsimd.memset(attn_all, 0.0)

            # ------------- edges + attention, chunks descending -------------

            def bcast8(ap2d, n):
                return ap2d.rearrange("p (c o) -> p c o", o=1).broadcast_to([128, n, H])

            def sim_block(t):
                n = NR[t]
                sl = slice(TOFF[t], TOFF[t + 1])
                # rinv = Exp(-0.5*Log(ms+eps)): Log/Exp/Square share one ACT
                # table set (Sqrt never co-resides with Exp -> table thrash)
                srt = wk.tile([128, 128], F32, tag="srt")
                nc.scalar.activation(out=srt[:, 0:n], in_=ss_all[:, sl],
                                     func=AF.Ln, bias=epsc, scale=1.0 / DIM)
                nc.scalar.activation(out=rinv_all[:, sl], in_=srt[:, 0:n],
                                     func=AF.Exp, scale=-0.5)
                nc.vector.tensor_mul(stgtmp[:, 0:n, :], stgraw[t],
                                     bcast8(rinv_all[:, sl], n))
                nc.vector.tensor_add(stgfin[t], stgtmp[:, 0:n, :],
                                     bcast8(mcol[:, sl], n))
                for h in range(H):
                    pb, ft = (h % 2) * 64, h // 2
                    sim_ps = sps.tile([128, 128], F32, tag="sim")
                    nc.tensor.matmul(sim_ps[:, 0:n],
                                     lhsT=kT[pb:pb + 64, ft, t * 128:(t + 1) * 128],
                                     rhs=qT[pb:pb + 64, ft, 32 * t:128],
                                     start=True, stop=True)
                    scr = scrp.tile([128, 128], F32, tag="scr")
                    nc.vector.tensor_add(scr[:, 0:n], sim_ps[:, 0:n],
                                         stgfin[t][:, :, h])
                    nc.scalar.activation(out=attn_all[:, t, h, 32 * t:128],
                                         in_=scr[:, 0:n],
                                         func=AF.Exp,
                                         bias=bedge[:, h:h + 1], scale=1.0)

            sig_of_q = []
            for c in range(NCH):
                m = [0] * 16
                for (t, sig0, qs, r0, cnt) in CHUNK_GROUPS[c]:
                    for i, q in enumerate(qs):
                        m[q] = sig0 + i
                sig_of_q.append(m)

            for c in range(NCH - 1, -1, -1):
                e16 = ep.tile([128, 16, DIM], BF16, tag="e16")
                nc.scalar.dma_start(
                    out=e16,
                    in_=ef_ext[2048 * c:2048 * (c + 1), :]
                    .rearrange("(q p) d -> p q d", p=128))
                # xbar transposes corrupt when two run concurrently — keep
                # them all on one queue (sync) so they serialize; sync does
                # nothing else during the edge stream.
                eTc = etp.tile([128, 32, 128], BF16, tag="eT")
                nc.sync.dma_start(out=eTc, in_=e16, transpose=True)
                bias_chunk = bps.tile([128, 16, H], F32, tag="bias")
                for q in range(16):
                    k = 16 * c + q
                    r, t = TILE_REV[k]
                    p = POS(r, t)
                    ss_eng = SS_PATTERN[k % 16]
                    if ss_eng == "a":
                        dump = sqp.tile([128, DIM], BF16, tag="sq")
                        nc.scalar.activation(out=dump, in_=e16[:, q, :],
                                             func=AF.Square,
                                             accum_out=ss_all[:, p:p + 1])
                    elif ss_eng == "v":
                        sq = sqp.tile([128, DIM], BF16, tag="sq")
                        nc.vector.scalar_tensor_tensor(
                            out=sq, in0=e16[:, q, :], scalar=1.0,
                            in1=e16[:, q, :], op0=MUL, op1=MUL,
                            accum_out=ss_all[:, p:p + 1])
                    else:
                        sq = sqp.tile([128, DIM], BF16, tag="sq")
                        nc.gpsimd.tensor_tensor(sq, e16[:, q, :], e16[:, q, :],
                                                op=MUL)
                        nc.vector.tensor_reduce(out=ss_all[:, p:p + 1], in_=sq,
                                                axis=mybir.AxisListType.X,
                                                op=ADD)
                    for kh in range(2):
                        nc.tensor.matmul(bias_chunk[:, sig_of_q[c][q], :],
                                         lhsT=eTc[:, 2 * q + kh, :],
                                         rhs=we16[:, kh, :],
                                         start=(kh == 0), stop=(kh == 1))
                for (t, sig0, qs, r0, cnt) in CHUNK_GROUPS[c]:
                    nc.vector.tensor_copy(
                        stgraw[t][:, r0 - 32 * t:r0 - 32 * t + cnt, :],
                        bias_chunk[:, sig0:sig0 + cnt, :])
                if c == 12:
                    sim_block(3)
                elif c == 6:
                    sim_block(2)
                elif c == 2:
                    sim_block(1)
            sim_block(0)

            # den/av accumulation: per region, all 4 t-mms back-to-back
            for h in range(H):
                for t in range(4):
                    nc.tensor.matmul(
                        den_ps[h // 4][0:1, (h % 4) * 128:(h % 4) * 128 + 128],
                        lhsT=ones16, rhs=attn_all[:, t, h, :],
                        start=(t == 0), stop=(t == 3))
            for h in range(H):
                for t in range(4):
                    nc.tensor.matmul(av_ps[:, h * DH:(h + 1) * DH],
                                     lhsT=attn_all[:, t, h, :],
                                     rhs=v16[:, t, h * DH:(h + 1) * DH],
                                     start=(t == 0), stop=(t == 3))

            # ---------------- epilogue ----------------
            den_sb = cst.tile([1, 2, 512], F32)
            for g in range(2):
                nc.vector.tensor_copy(den_sb[:, g, :], den_ps[g])
            denT_ps = mps.tile([128, H], F32, tag="setup_ps")
            for h in range(H):
                nc.tensor.matmul(denT_ps[:, h:h + 1],
                                 lhsT=den_sb[0:1, h // 4, (h % 4) * 128:(h % 4) * 128 + 128],
                                 rhs=onef, start=True, stop=True)
            rv = cst.tile([128, H], F32)
            nc.vector.reciprocal(rv, denT_ps)
            av_sb = cst.tile([128, H * DH], BF16)
            for h in range(H):
                nc.vector.tensor_scalar(out=av_sb[:, h * DH:(h + 1) * DH],
                                        in0=av_ps[:, h * DH:(h + 1) * DH],
                                        scalar1=rv[:, h:h + 1], scalar2=None,
                                        op0=MUL)
            avT = cst.tile([128, 4, 128], BF16)
            nc.sync.dma_start(out=avT, in_=av_sb, transpose=True)
            out_ps = avps.tile([128, DIM], F32, tag="avout")
            for q4 in range(4):
                nc.tensor.matmul(out_ps, lhsT=avT[:, q4, :], rhs=wo16[:, q4, :],
                                 start=(q4 == 0), stop=(q4 == 3))
            out_sb = cst.tile([128, DIM], F32)
            nc.vector.tensor_copy(out_sb, out_ps)
            nc.sync.dma_start(out=out_ext[:, :], in_=out_sb)
            if debug:
                for t in range(4):
                    nc.sync.dma_start(out=dbg_stg[t][:, :, :], in_=stgfin[t])
                    nc.sync.dma_start(out=dbg_raw[t][:, :, :], in_=stgraw[t])
                nc.sync.dma_start(out=dbg_ss[:, :], in_=ss_all)
                nc.sync.dma_start(out=dbg_rinv[:, :], in_=rinv_all)
                nc.sync.dma_start(out=dbg_den[:, :, :], in_=den_sb)
                av32 = cst.tile([128, H * DH], F32)
                nc.vector.tensor_copy(av32, av_sb)
                nc.sync.dma_start(out=dbg_av[:, :], in_=av32)
                nc.sync.dma_start(out=dbg_kT[:, :, :], in_=kT)
                nc.sync.dma_start(out=dbg_qT[:, :, :], in_=qT)
    return nc


_NC_CACHE = [None]
LAST_RESULT = [None]


def kernel(x, mask, edges, gamma_x, W_qkv, gamma_e, W_edge, b_edge, W_out):
    x = np.asarray(x, np.float32)
    mask = np.asarray(mask)
    edges = np.asarray(edges, np.float32)
    gamma_x = np.asarray(gamma_x, np.float32)
    W_qkv = np.asarray(W_qkv, np.float32)
    gamma_e = np.asarray(gamma_e, np.float32)
    W_edge = np.asarray(W_edge, np.float32)
    b_edge = np.asarray(b_edge, np.float32)
    W_out = np.asarray(W_out, np.float32)

    wqkv_f = (gamma_x[:, None] * W_qkv).copy()
    wqkv_f[:, :H * DH] *= DH ** 0.5
    wq32 = np.ascontiguousarray(
        wqkv_f[:, :1024].reshape(2, 128, 8, 128).transpose(1, 0, 2, 3))
    wv16 = np.ascontiguousarray(
        wqkv_f[:, 1024:1536].reshape(2, 128, 4, 128).transpose(1, 0, 2, 3)
    ).astype(BF)
    wedge_f = gamma_e[:, None] * W_edge
    we16 = np.ascontiguousarray(
        wedge_f.reshape(2, 128, H).transpose(1, 0, 2)).astype(BF)
    wo16 = np.ascontiguousarray(
        W_out.reshape(4, 128, DIM).transpose(1, 0, 2)).astype(BF)
    ident = np.eye(128, dtype=BF)
    bedgec = np.ascontiguousarray(np.broadcast_to(b_edge, (128, H))).astype(np.float32)

    e_bf = edges.astype(BF)

    jj_all = np.arange(128)
    in_maps = []
    for c in range(8):
        b, l = c // 4, c % 4
        rows = []
        mcol = np.empty((128, NT), np.float32)
        for r in range(128):
            jt = r // 32 + 1
            rows.append(e_bf[b, 4 * r + l, :jt * 128, :])
            for t in range(jt):
                j = t * 128 + jj_all
                valid = (j <= 4 * r + l) & mask[b, j]
                mcol[:, POS(r, t)] = np.where(valid, 0.0, NEG)
        ef = np.ascontiguousarray(np.concatenate(rows, axis=0))
        xq = np.ascontiguousarray(x[b, l::4])
        in_maps.append({
            "ef": ef, "xb": x[b], "xq": xq,
            "wq32": wq32, "wv16": wv16, "we16": we16, "wo16": wo16,
            "mcol": mcol, "bedgec": bedgec, "ident": ident,
        })

    if _NC_CACHE[0] is None:
        nc = build()
        split_waits(nc)
        _NC_CACHE[0] = nc
    res = run_bass_kernel_spmd(_NC_CACHE[0], in_maps, core_ids=list(range(8)))
    LAST_RESULT[0] = res

    out = np.zeros((B, N, DIM), np.float32)
    for c in range(8):
        b, l = c // 4, c % 4
        out[b, l::4] = res.results[c]["out"]
    return out



# revision 6
# speedup vs baseline: 1.6421x; 1.6421x over previous
"""Trainium2 Bass kernel v3 for nn_Attention_22050362097758 (edge-biased causal
attention; b=2, n=512, dim=256, heads=8, dim_head=64).

Sharding: core c -> batch c//4, lane l=c%4, query rows i = 4r+l (r=0..127).
Row-interleaving makes causal extents core-uniform (SPMD: one program).

v3 changes vs v2: the on-device xbar transpose (21 MB SBUF<->SBUF, serialized)
is gone -- the host packs the causal-prefix edges TWICE in fp8e4 (10.5 MB
j-on-partitions for the sum-of-squares, 10.5 MB d-on-partitions for the bias
matmul), one 1 MB DMA per 16-tile chunk. Bias matmul keeps eT as fp8 weights
(FWL 4x weight load). b_edge + causal/key mask live in one host tensor mcolh;
the bias+mask adds ride the PE as identity-matmul deposits into the sim PSUM
accumulation group (start=True bank-clear discipline). Softmax exp is one
batched ACT op per j-block over all 8 heads. ss is split chunk-wise across
ACT (batched Square + DVE segmented tensor_reduce), GpSimd (fused
scalar_tensor_tensor, idle in v2), and DVE (fused stt). den/av accumulation
chains run in-stream on dedicated PSUM banks right after each sim block.
"""
import sys
sys.path.insert(0, "/opt/trn_rl_repo")
import numpy as np
import ml_dtypes

import concourse.bass as bass
import concourse.mybir as mybir
import concourse.tile as tile
from concourse.bass_utils import run_bass_kernel_spmd

B, N, DIM = 2, 512, 256
H, DH = 8, 64
EPS = 1e-5
NEG = -1e30
F32 = mybir.dt.float32
BF16 = mybir.dt.bfloat16
FP8 = mybir.dt.float8e4
BF = ml_dtypes.bfloat16
F8 = ml_dtypes.float8_e4m3fn

# Tiles enumerated ascending (r, t) -- matches the DRAM packing order.
# Chunks emitted descending (high rows first) so sim j-block t can start as
# soon as rows >= 32t are staged.
TILE_IDX = {}
TILE_REV = []
for _r in range(128):
    for _t in range(_r // 32 + 1):
        TILE_IDX[(_r, _t)] = len(TILE_REV)
        TILE_REV.append((_r, _t))
NT = len(TILE_REV)   # 320
NCH = NT // 16       # 20 chunks

# ss/mcolh columns laid out t-grouped: block t occupies columns
# [TOFF[t], TOFF[t+1]) indexed by (r - 32t).
TOFF = [0, 128, 224, 288, 320]
NR = [128, 96, 64, 32]
def POS(r, t):
    return TOFF[t] + (r - 32 * t)

# per-chunk group structure: groups[t] = (t, sig0, q_list, r0, cnt)
CHUNK_GROUPS = []
for _c in range(NCH):
    by_t = {}
    for _q in range(16):
        _r, _t = TILE_REV[16 * _c + _q]
        by_t.setdefault(_t, []).append((_q, _r))
    groups = []
    sig = 0
    for _t in sorted(by_t):
        qs = by_t[_t]
        r0 = qs[0][1]
        assert [r for _, r in qs] == list(range(r0, r0 + len(qs)))
        groups.append((_t, sig, [q for q, _ in qs], r0, len(qs)))
        sig += len(qs)
    CHUNK_GROUPS.append(groups)

# per-chunk ss engine: 'a'=ACT batched Square + DVE segmented reduce,
# 'g'=GpSimd fused stt, 'v'=DVE fused stt.  (indexed by chunk id)
SS_MODE = "gagavagagvagagvagagv"
assert len(SS_MODE) == NCH

_ctr = [0]


def _nop_with_wait(engine, wait):
    _ctr[0] += 1
    n = mybir.InstNoOp.__new__(mybir.InstNoOp, name=f"waitnop-{_ctr[0]}")
    n.engine = engine
    n.sync_info = mybir.SyncInfo.__new__(mybir.SyncInfo, on_wait=[wait], on_update=[])
    return n


def split_waits(nc):
    """Walrus encodes at most ONE sem-wait per instruction; Tile attaches
    many. Move extras onto NOPs inserted just before, same engine."""
    for f in nc.m.functions:
        for b in f.blocks:
            out, changed = [], False
            for inst in b.instructions:
                si = inst.sync_info
                waits = list(si.on_wait) if (si and si.on_wait) else []
                keep = 0 if inst.opcode == "Drain" else 1
                if len(waits) > keep:
                    changed = True
                    moved = waits[:-keep] if keep else waits
                    kept = waits[-keep:] if keep else []
                    for w in moved:
                        out.append(_nop_with_wait(inst.engine, w))
                    inst.sync_info = mybir.SyncInfo.__new__(
                        mybir.SyncInfo, on_wait=kept,
                        on_update=list(si.on_update) if si.on_update else [])
                out.append(inst)
            if changed:
                b.instructions = out


def build(debug=False):
    nc = bass.Bass()
    ef8_ext = nc.declare_dram_parameter("ef8", [128, NCH, 2, 16, DIM], FP8,
                                        isOutput=False)
    x_ext = nc.declare_dram_parameter("xb", [N, DIM], F32, isOutput=False)
    xq_ext = nc.declare_dram_parameter("xq", [128, DIM], F32, isOutput=False)
    wq32_ext = nc.declare_dram_parameter("wq32", [128, 2, 8, 128], F32, isOutput=False)
    wv16_ext = nc.declare_dram_parameter("wv16", [128, 2, 4, 128], BF16, isOutput=False)
    we16_ext = nc.declare_dram_parameter("we16", [128, 2, H], BF16, isOutput=False)
    wo16_ext = nc.declare_dram_parameter("wo16", [128, 4, DIM], BF16, isOutput=False)
    mcolh_ext = nc.declare_dram_parameter("mcolh", [128, NT, H], BF16, isOutput=False)
    id_ext = nc.declare_dram_parameter("ident", [128, 128], BF16, isOutput=False)
    out_ext = nc.declare_dram_parameter("out", [128, DIM], F32, isOutput=True)
    if debug:
        dbg_ss = nc.declare_dram_parameter("dbg_ss", [128, NT], F32, isOutput=True)
        dbg_raw = [nc.declare_dram_parameter(f"dbg_raw{t}", [128, NR[t], H], F32,
                                             isOutput=True) for t in range(4)]
        dbg_den = nc.declare_dram_parameter("dbg_den", [1, 2, 512], F32, isOutput=True)
        dbg_av = nc.declare_dram_parameter("dbg_av", [128, H * DH], F32, isOutput=True)
        dbg_attn = nc.declare_dram_parameter("dbg_attn", [128, 4, H, 128], F32,
                                             isOutput=True)

    AF = mybir.ActivationFunctionType
    MUL, ADD = mybir.AluOpType.mult, mybir.AluOpType.add

    with tile.TileContext(nc) as tc:
        with tc.tile_pool(name="cst", bufs=1) as cst, \
             tc.tile_pool(name="ep", bufs=3) as ep, \
             tc.tile_pool(name="sqp", bufs=2) as sqp, \
             tc.tile_pool(name="dvp", bufs=2) as dvp, \
             tc.tile_pool(name="dgp", bufs=2) as dgp, \
             tc.tile_pool(name="wk", bufs=2) as wk, \
             tc.tile_pool(name="bps", bufs=2, space="PSUM") as bps, \
             tc.tile_pool(name="sps", bufs=1, space="PSUM") as sps, \
             tc.tile_pool(name="dps", bufs=1, space="PSUM") as dps, \
             tc.tile_pool(name="mps", bufs=1, space="PSUM") as mps, \
             tc.tile_pool(name="avps", bufs=1, space="PSUM") as avps:

            # ---------------- constants ----------------
            ident = cst.tile([128, 128], BF16)
            nc.sync.dma_start(out=ident, in_=id_ext[:, :])
            we16 = cst.tile([128, 2, H], BF16)
            nc.sync.dma_start(out=we16, in_=we16_ext[:, :, :])
            wo16 = cst.tile([128, 4, DIM], BF16)
            nc.sync.dma_start(out=wo16, in_=wo16_ext[:, :, :])
            wq32 = cst.tile([128, 2, 8, 128], F32)
            nc.sync.dma_start(out=wq32, in_=wq32_ext[:, :, :, :])
            wv16 = cst.tile([128, 2, 4, 128], BF16)
            nc.sync.dma_start(out=wv16, in_=wv16_ext[:, :, :, :])
            mcolh = cst.tile([128, NT, H], BF16)
            nc.sync.dma_start(out=mcolh, in_=mcolh_ext[:, :, :])
            x32 = cst.tile([128, 5, DIM], F32)
            nc.sync.dma_start(out=x32[:, 0:4, :],
                              in_=x_ext.rearrange("(t p) d -> p t d", p=128))
            nc.sync.dma_start(out=x32[:, 4, :], in_=xq_ext[:, :])

            epsc = cst.tile([128, 1], F32)
            nc.vector.memset(epsc, EPS)
            ones16 = cst.tile([128, 1], BF16)
            nc.vector.memset(ones16, 1.0)
            onef = cst.tile([1, 1], F32)
            nc.vector.memset(onef, 1.0)

            ss_all = cst.tile([128, NT], F32)
            stgraw = []
            for t in range(4):
                sr = cst.tile([128, NR[t], H], BF16, tag=f"sraw{t}", name=f"sraw{t}")
                stgraw.append(sr)

            # ---------------- x path: rmsnorm, kT, qT, v ----------------
            ssx = wk.tile([128, 5], F32, tag="ssx")
            for t in range(5):
                dump = wk.tile([128, DIM], BF16, tag="sqdump")
                nc.scalar.activation(out=dump, in_=x32[:, t, :],
                                     func=AF.Square, accum_out=ssx[:, t:t + 1])
            sqm = wk.tile([128, 5], F32, tag="sqm")
            nc.scalar.activation(out=sqm, in_=ssx, func=AF.Ln,
                                 bias=epsc, scale=1.0 / DIM)
            rx = wk.tile([128, 5], F32, tag="rx")
            nc.scalar.activation(out=rx, in_=sqm, func=AF.Exp, scale=-0.5)
            xn32 = cst.tile([128, 5, DIM], F32)
            for t in range(5):
                nc.vector.tensor_scalar(out=xn32[:, t, :], in0=x32[:, t, :],
                                        scalar1=rx[:, t:t + 1], scalar2=None,
                                        op0=MUL)
            ident32 = cst.tile([128, 128], F32)
            nc.vector.tensor_copy(ident32, ident)
            xnT32 = cst.tile([128, 2, 5, 128], F32)
            for t in range(5):
                ps32 = mps.tile([128, 2, 128], F32, tag="setup_ps")
                for kh in range(2):
                    nc.tensor.transpose(ps32[:, kh, :],
                                        xn32[:, t, kh * 128:(kh + 1) * 128], ident32)
                nc.vector.tensor_copy(xnT32[:, :, t, :], ps32)
            xnT = cst.tile([128, 2, 5, 128], BF16)
            nc.vector.tensor_copy(xnT, xnT32)

            kT = cst.tile([128, 4, N], BF16)
            for ft in range(4):
                k_ps = mps.tile([128, N], F32, tag="setup_ps")
                for kh in range(2):
                    nc.tensor.matmul(k_ps,
                                     lhsT=wq32[:, kh, 4 + ft, :],
                                     rhs=xnT32[:, kh, 0:4, :].rearrange("p a b -> p (a b)"),
                                     start=(kh == 0), stop=(kh == 1))
                nc.scalar.copy(kT[:, ft, :], k_ps)
            qT = cst.tile([128, 4, 128], BF16)
            for ft in range(4):
                q_ps = mps.tile([128, 128], F32, tag="setup_ps")
                for kh in range(2):
                    nc.tensor.matmul(q_ps, lhsT=wq32[:, kh, ft, :],
                                     rhs=xnT32[:, kh, 4, :],
                                     start=(kh == 0), stop=(kh == 1))
                nc.scalar.copy(qT[:, ft, :], q_ps)
            v16 = cst.tile([128, 4, H * DH], BF16)
            for st in range(4):
                v_ps = mps.tile([128, H * DH], F32, tag="setup_ps")
                for kh in range(2):
                    nc.tensor.matmul(v_ps,
                                     lhsT=xnT[:, kh, st, :],
                                     rhs=wv16[:, kh, :, :].rearrange("p a b -> p (a b)"),
                                     start=(kh == 0), stop=(kh == 1))
                nc.scalar.copy(v16[:, st, :], v_ps)

            av_ps = avps.tile([128, H * DH], F32, tag="avout")
            den_ps = []
            for g in range(2):
                dtile = dps.tile([1, 512], F32, tag=f"den{g}", name=f"den_ps{g}")
                den_ps.append(dtile)
            # attn tiles: [j, t, h, i]; masked region stays 0 from this memset
            attn_all = cst.tile([128, 4, H, 128], BF16)
            nc.gpsimd.memset(attn_all, 0.0)

            # ------------- edges + attention, chunks descending -------------

            def bcast8(ap2d, n):
                return ap2d.rearrange("p (c o) -> p c o", o=1).broadcast_to([128, n, H])

            def sim_block(t):
                n = NR[t]
                sl = slice(TOFF[t], TOFF[t + 1])
                # rinv = Exp(-0.5*Log(ms+eps)): Ln/Exp/Square share one ACT
                # table set
                srt = wk.tile([128, 128], F32, tag="srt")
                nc.scalar.activation(out=srt[:, 0:n], in_=ss_all[:, sl],
                                     func=AF.Ln, bias=epsc, scale=1.0 / DIM)
                rinv = wk.tile([128, 128], F32, tag="rinv")
                nc.scalar.activation(out=rinv[:, 0:n], in_=srt[:, 0:n],
                                     func=AF.Exp, scale=-0.5)
                stgtmp = wk.tile([128, 128, H], BF16, tag="stgtmp")
                nc.vector.tensor_mul(stgtmp[:, 0:n, :], stgraw[t],
                                     bcast8(rinv[:, 0:n], n))
                sim_all = sps.tile([128, H, 128], F32, tag="sim")
                for h in range(H):
                    pb, ft = (h % 2) * 64, h // 2
                    nc.tensor.matmul(sim_all[:, h, 0:n], lhsT=ident,
                                     rhs=stgtmp[:, 0:n, h],
                                     start=(h % 4 == 0), stop=False)
                    nc.tensor.matmul(sim_all[:, h, 0:n], lhsT=ident,
                                     rhs=mcolh[:, sl, h],
                                     start=False, stop=False)
                    nc.tensor.matmul(sim_all[:, h, 0:n],
                                     lhsT=kT[pb:pb + 64, ft, t * 128:(t + 1) * 128],
                                     rhs=qT[pb:pb + 64, ft, 32 * t:128],
                                     start=False, stop=True)
                nc.scalar.activation(out=attn_all[:, t, :, 32 * t:],
                                     in_=sim_all[:, :, 0:n], func=AF.Exp)
                for h in range(H):
                    nc.tensor.matmul(
                        den_ps[h // 4][0:1, (h % 4) * 128:(h % 4) * 128 + 128],
                        lhsT=ones16, rhs=attn_all[:, t, h, :],
                        start=(t == 3 and h % 4 == 0), stop=(t == 0))
                for h in range(H):
                    nc.tensor.matmul(av_ps[:, h * DH:(h + 1) * DH],
                                     lhsT=attn_all[:, t, h, :],
                                     rhs=v16[:, t, h * DH:(h + 1) * DH],
                                     start=(t == 3 and h == 0), stop=(t == 0))

            sig_of_q = []
            for c in range(NCH):
                m = [0] * 16
                for (t, sig0, qs, r0, cnt) in CHUNK_GROUPS[c]:
                    for i, q in enumerate(qs):
                        m[q] = sig0 + i
                sig_of_q.append(m)

            for c in range(NCH - 1, -1, -1):
                ec = ep.tile([128, 2, 16, DIM], FP8, tag="ec")
                nc.scalar.dma_start(out=ec, in_=ef8_ext[:, c, :, :, :])
                mode = SS_MODE[c]
                bias_chunk = bps.tile([128, 16, H], F32, tag="bias")
                if mode in ("a", "g"):
                    sq16 = sqp.tile([128, 16, DIM], BF16, tag="sq16")
                    if mode == "a":
                        nc.scalar.activation(out=sq16, in_=ec[:, 0, :, :],
                                             func=AF.Square)
                    else:
                        nc.gpsimd.tensor_tensor(sq16, ec[:, 0, :, :],
                                                ec[:, 0, :, :], op=MUL)
                    for (t, sig0, qs, r0, cnt) in CHUNK_GROUPS[c]:
                        p0 = POS(r0, t)
                        nc.vector.tensor_reduce(
                            out=ss_all[:, p0:p0 + cnt],
                            in_=sq16[:, sig0:sig0 + cnt, :],
                            axis=mybir.AxisListType.X, op=ADD)
                for q in range(16):
                    k = 16 * c + q
                    r, t = TILE_REV[k]
                    p = POS(r, t)
                    if mode == "v":
                        dmp = dvp.tile([128, DIM], BF16, tag="dv")
                        nc.vector.scalar_tensor_tensor(
                            out=dmp, in0=ec[:, 0, q, :], scalar=1.0,
                            in1=ec[:, 0, q, :], op0=MUL, op1=MUL,
                            accum_out=ss_all[:, p:p + 1])
                    for kh in range(2):
                        nc.tensor.matmul(bias_chunk[:, sig_of_q[c][q], :],
                                         lhsT=ec[:, 1, q, kh * 128:(kh + 1) * 128],
                                         rhs=we16[:, kh, :],
                                         start=(kh == 0), stop=(kh == 1))
                for (t, sig0, qs, r0, cnt) in CHUNK_GROUPS[c]:
                    nc.vector.tensor_copy(
                        stgraw[t][:, r0 - 32 * t:r0 - 32 * t + cnt, :],
                        bias_chunk[:, sig0:sig0 + cnt, :])
                if c == 12:
                    sim_block(3)
                elif c == 6:
                    sim_block(2)
                elif c == 2:
                    sim_block(1)
            sim_block(0)

            # ---------------- epilogue ----------------
            den_sb = cst.tile([1, 2, 512], F32)
            for g in range(2):
                nc.vector.tensor_copy(den_sb[:, g, :], den_ps[g])
            denT_ps = mps.tile([128, H], F32, tag="setup_ps")
            for h in range(H):
                nc.tensor.matmul(denT_ps[:, h:h + 1],
                                 lhsT=den_sb[0:1, h // 4, (h % 4) * 128:(h % 4) * 128 + 128],
                                 rhs=onef, start=True, stop=True)
            rv = cst.tile([128, H], F32)
            nc.vector.reciprocal(rv, denT_ps)
            av_sb = cst.tile([128, H * DH], BF16)
            for h in range(H):
                nc.vector.tensor_scalar(out=av_sb[:, h * DH:(h + 1) * DH],
                                        in0=av_ps[:, h * DH:(h + 1) * DH],
                                        scalar1=rv[:, h:h + 1], scalar2=None,
                                        op0=MUL)
            avT = cst.tile([128, 4, 128], BF16)
            nc.sync.dma_start(out=avT, in_=av_sb, transpose=True)
            out_ps = avps.tile([128, DIM], F32, tag="avout")
            for q4 in range(4):
                nc.tensor.matmul(out_ps, lhsT=avT[:, q4, :], rhs=wo16[:, q4, :],
                                 start=(q4 == 0), stop=(q4 == 3))
            out_sb = cst.tile([128, DIM], F32)
            nc.vector.tensor_copy(out_sb, out_ps)
            nc.sync.dma_start(out=out_ext[:, :], in_=out_sb)
            if debug:
                nc.sync.dma_start(out=dbg_ss[:, :], in_=ss_all)
                for t in range(4):
                    raw32 = cst.tile([128, NR[t], H], F32, tag=f"r32_{t}",
                                     name=f"r32_{t}")
                    nc.vector.tensor_copy(raw32, stgraw[t])
                    nc.sync.dma_start(out=dbg_raw[t][:, :, :], in_=raw32)
                nc.sync.dma_start(out=dbg_den[:, :, :], in_=den_sb)
                av32 = cst.tile([128, H * DH], F32)
                nc.vector.tensor_copy(av32, av_sb)
                nc.sync.dma_start(out=dbg_av[:, :], in_=av32)
                at32 = cst.tile([128, 4, H, 128], F32)
                nc.vector.tensor_copy(at32, attn_all)
                nc.sync.dma_start(out=dbg_attn[:, :, :, :], in_=at32)
    return nc


_NC_CACHE = [None]
LAST_RESULT = [None]


def _pack_core(edges_b8, x, b, l, mask, b_edge):
    """Per-core host packing: fp8 causal-prefix edges in two layouts + mcolh."""
    E8 = edges_b8[l::4]                       # [128, 512, 256] fp8
    e8_parts, eT_parts = [], []
    for r in range(128):
        jt = r // 32 + 1
        A = E8[r, :jt * 128, :]               # [jt*128, 256]
        e8_parts.append(A.reshape(jt, 128, DIM).transpose(1, 0, 2))
        AT = np.ascontiguousarray(A.T)        # [256, jt*128]
        eT_parts.append(AT.reshape(2, 128, jt, 128).transpose(1, 2, 0, 3))
    e8_all = np.concatenate(e8_parts, axis=1)     # [128, NT, 256]
    eT_all = np.concatenate(eT_parts, axis=1)     # [128, NT, 2, 128]
    ef8 = np.empty((128, NCH, 2, 16, DIM), dtype=F8)
    ef8[:, :, 0] = e8_all.reshape(128, NCH, 16, DIM)
    ef8[:, :, 1] = eT_all.reshape(128, NCH, 16, DIM)

    jj = np.arange(128)
    mcolh = np.empty((128, NT, H), np.float32)
    for r in range(128):
        for t in range(r // 32 + 1):
            j = t * 128 + jj
            valid = (j <= 4 * r + l) & mask[b, j]
            mcolh[:, POS(r, t), :] = np.where(valid[:, None], b_edge[None, :], NEG)
    xq = np.ascontiguousarray(x[b, l::4])
    return ef8, mcolh.astype(BF), xq


def kernel(x, mask, edges, gamma_x, W_qkv, gamma_e, W_edge, b_edge, W_out):
    x = np.asarray(x, np.float32)
    mask = np.asarray(mask)
    edges = np.asarray(edges, np.float32)
    gamma_x = np.asarray(gamma_x, np.float32)
    W_qkv = np.asarray(W_qkv, np.float32)
    gamma_e = np.asarray(gamma_e, np.float32)
    W_edge = np.asarray(W_edge, np.float32)
    b_edge = np.asarray(b_edge, np.float32)
    W_out = np.asarray(W_out, np.float32)

    wqkv_f = (gamma_x[:, None] * W_qkv).copy()
    wqkv_f[:, :H * DH] *= DH ** 0.5
    wq32 = np.ascontiguousarray(
        wqkv_f[:, :1024].reshape(2, 128, 8, 128).transpose(1, 0, 2, 3))
    wv16 = np.ascontiguousarray(
        wqkv_f[:, 1024:1536].reshape(2, 128, 4, 128).transpose(1, 0, 2, 3)
    ).astype(BF)
    wedge_f = gamma_e[:, None] * W_edge
    we16 = np.ascontiguousarray(
        wedge_f.reshape(2, 128, H).transpose(1, 0, 2)).astype(BF)
    wo16 = np.ascontiguousarray(
        W_out.reshape(4, 128, DIM).transpose(1, 0, 2)).astype(BF)
    ident = np.eye(128, dtype=BF)

    edges8 = np.clip(edges, -224.0, 224.0).astype(F8)   # [2, 512, 512, 256]

    in_maps = []
    for c in range(8):
        b, l = c // 4, c % 4
        ef8, mcolh, xq = _pack_core(edges8[b], x, b, l, mask, b_edge)
        in_maps.append({
            "ef8": ef8, "xb": x[b], "xq": xq,
            "wq32": wq32, "wv16": wv16, "we16": we16, "wo16": wo16,
            "mcolh": mcolh, "ident": ident,
        })

    if _NC_CACHE[0] is None:
        nc = build()
        split_waits(nc)
        _NC_CACHE[0] = nc
    res = run_bass_kernel_spmd(_NC_CACHE[0], in_maps, core_ids=list(range(8)))
    LAST_RESULT[0] = res

    out = np.zeros((B, N, DIM), np.float32)
    for c in range(8):
        b, l = c // 4, c % 4
        out[b, l::4] = res.results[c]["out"]
    return out
